# revision 14
# baseline (speedup 1.0000x reference)
"""Trainium2 Bass kernel for nn_NeuralODE: GRU encoder -> h2o MLP -> Tsit5
neural-ODE solve (causal-flow vector field) -> o2d MLP.

Sharding: data-parallel over batch B=256 across 8 cores (32 samples/core),
weights replicated. All recurrent compute in bf16 with an fp32 master copy
of the ODE state; activations/gates transposed so features sit on SBUF
partitions ([feat, batch] layout) and batch is the matmul moving dim.
"""

import os
import sys
import types

import numpy as np
import ml_dtypes

import concourse.bass as bass
import concourse.mybir as mybir
from concourse.bass import ds
from concourse.tile import TileContext
from concourse.vector_clock import ScopedClock
from concourse import bass_utils

BF16 = mybir.dt.bfloat16
F32 = mybir.dt.float32
AF = mybir.ActivationFunctionType
OP = mybir.AluOpType

B, T, D = 256, 256, 16
H, WD, ODE, S = 128, 256, 32, 8
NC = 8
BL = B // NC  # 32 samples per core

A21 = 0.161
A31, A32 = -0.008480655492356989, 0.335480655492357
A41, A42, A43 = 2.8971530571054935, -6.359448489975075, 4.3622954328695815
A51, A52, A53, A54 = 5.325864828439257, -11.748883564062828, 7.4955393428898365, -0.09249506636175525
A61, A62, A63, A64, A65 = 5.86145544294642, -12.92096931784711, 8.159367898576159, -0.071584973281401, -0.028269050394068383
B1, B2, B3, B4, B5, B6 = 0.09646076681806523, 0.01, 0.4798896504144996, 1.379008574103742, -3.290069515436081, 2.324710524099774

GRU_UNROLL = 16
ODE_UNROLL = 15

# ---------------------------------------------------------------------------
# Workarounds for this container's walrus build (max 1 sem-wait per
# instruction): split the TileContext tail drain, and hoist extra waits on
# every instruction onto same-engine no-ops.
# ---------------------------------------------------------------------------


def _patched_drain_and_barrier(self, tick_clock, wait_clock):
    nc = self.nc
    drain_inst = nc.sync.drain()
    wait_clock.add_sem_waits(drain_inst.ins, ScopedClock({None: tick_clock.global_clock}))
    si = drain_inst.ins.sync_info
    if si is not None and si.on_wait is not None and len(si.on_wait) > 1:
        waits = list(si.on_wait)
        del si.on_wait[1:]
        for w in waits[1:]:
            d2 = nc.sync.drain()
            if d2.ins.sync_info is None:
                d2.ins.sync_info = mybir.SyncInfo(on_wait=[w], on_update=[])
            else:
                d2.ins.sync_info.on_wait.append(w)
    nc.all_engine_barrier()
    assert self.sems is not None
    popped = nc._tile_sem_poison_stack.pop()
    assert popped is self._sem_poison
    nc.clear_and_free_semaphores(list(self.sems.allocated().values()))
    nc.all_engine_barrier()


def _apply_tile_patch():
    import concourse.tile as tile_mod

    tile_mod.TileContext._drain_and_barrier = _patched_drain_and_barrier


def _split_waits(nc):
    for fn in nc.m.functions:
        for bb in fn.blocks:
            new_insts = []
            for inst in bb.instructions:
                si = inst.sync_info
                if si is not None and si.on_wait is not None and len(si.on_wait) > 1:
                    waits = list(si.on_wait)
                    keep = waits[-1]
                    for i, w in enumerate(waits[:-1]):
                        nop = mybir.InstNoOp(
                            name=f"{inst.name}_wsplit{i}",
                            engine=inst.engine,
                            sync_info=mybir.SyncInfo(on_wait=[w], on_update=[]),
                            bass_nofuse=True,
                        )
                        new_insts.append(nop)
                    del si.on_wait[:]
                    si.on_wait.append(keep)
                new_insts.append(inst)
            bb.instructions[:] = new_insts


_apply_tile_patch()


# ---------------------------------------------------------------------------
# Bass program (identical on every core; per-core data differs via in_maps)
# ---------------------------------------------------------------------------


def build_program():
    nc = bass.Bass("TRN2", num_devices=NC)

    def din(name, shape, dt=BF16):
        return nc.dram_tensor(name, shape, dt, kind="ExternalInput")

    xT = din("xT", [D, T * BL])                  # time-reversed input, [d, (t', b)]
    wihT = din("wihT", [D, 3 * H])
    whhT = din("whhT", [H, 3 * H])
    gbias = din("gbias", [H, 3], F32)            # r,z,c input-bias columns
    gbn = din("gbn", [H, 1], F32)
    h2oW1T = din("h2oW1T", [H, 2 * H])
    h2ob1 = din("h2ob1", [H, 2], F32)
    h2oW2T = din("h2oW2T", [H, 4 * H])           # K0M0,K0M1,K1M0,K1M1
    h2ob2 = din("h2ob2", [H, 2], F32)
    h2oW3T = din("h2oW3T", [H, 2 * ODE])         # K0,K1
    h2ob3 = din("h2ob3", [ODE, 1], F32)
    c1 = din("c1", [ODE, 256])
    c2 = din("c2", [2 * ODE, 256])
    c3 = din("c3", [3 * ODE, 256])
    c4 = din("c4", [4 * ODE, 256])
    c5a = din("c5a", [4 * ODE, 256])
    c5b = din("c5b", [ODE, 256])
    c6a = din("c6a", [4 * ODE, 256])
    c6b = din("c6b", [2 * ODE, 256])
    w1cT = din("w1cT", [H, 3 * H])               # T00,T10,T11 (lower-tri of W1.T)
    w2cT = din("w2cT", [H, 3 * H])
    woT = din("woT", [H, 2 * ODE])               # Wout.T K-tiles
    bd1 = din("bd1", [4 * ODE, ODE])             # [0;B1;B2;B3] x I
    bd2 = din("bd2", [3 * ODE, ODE])             # [B4;B5;0] x I
    bd2l = din("bd2l", [3 * ODE, ODE])           # [0;0;B6] x I
    dtb = din("dtb", [H, T - 1], F32)            # dt broadcast to partitions
    o2dW1T = din("o2dW1T", [ODE, 256])
    o2db1 = din("o2db1", [H, 2], F32)
    o2dW2T = din("o2dW2T", [H, 4 * H])
    o2db2 = din("o2db2", [H, 2], F32)
    o2dW3T = din("o2dW3T", [H, 2 * D])
    o2db3 = din("o2db3", [D, 1], F32)
    outT = nc.dram_tensor("outT", [D, T * BL], F32, kind="ExternalOutput")

    mm = nc.tensor.matmul

    with TileContext(nc) as tc:
        with tc.tile_pool(name="wp", bufs=1) as wp:
            def lw(name, dram, dt=None):
                t = wp.tile(list(dram.shape), dt or dram.dtype, name=name, tag=name)
                nc.sync.dma_start(out=t[:], in_=dram[:])
                return t

            s_xT = lw("s_xT", xT)
            s_wihT = lw("s_wihT", wihT)
            s_whhT = lw("s_whhT", whhT)
            s_gbias = lw("s_gbias", gbias)
            s_gbn = lw("s_gbn", gbn)
            s_h2oW1T = lw("s_h2oW1T", h2oW1T)
            s_h2ob1 = lw("s_h2ob1", h2ob1)
            s_h2oW2T = lw("s_h2oW2T", h2oW2T)
            s_h2ob2 = lw("s_h2ob2", h2ob2)
            s_h2oW3T = lw("s_h2oW3T", h2oW3T)
            s_h2ob3 = lw("s_h2ob3", h2ob3)
            s_c = {1: lw("s_c1", c1), 2: lw("s_c2", c2), 3: lw("s_c3", c3),
                   4: lw("s_c4", c4)}
            s_c5a, s_c5b = lw("s_c5a", c5a), lw("s_c5b", c5b)
            s_c6a, s_c6b = lw("s_c6a", c6a), lw("s_c6b", c6b)
            s_w1cT = lw("s_w1cT", w1cT)
            s_w2cT = lw("s_w2cT", w2cT)
            s_woT = lw("s_woT", woT)
            s_bd1, s_bd2 = lw("s_bd1", bd1), lw("s_bd2", bd2)
            s_bd2l = lw("s_bd2l", bd2l)
            s_dtb = lw("s_dtb", dtb)
            s_o2dW1T = lw("s_o2dW1T", o2dW1T)
            s_o2db1 = lw("s_o2db1", o2db1)
            s_o2dW2T = lw("s_o2dW2T", o2dW2T)
            s_o2db2 = lw("s_o2db2", o2db2)
            s_o2dW3T = lw("s_o2dW3T", o2dW3T)
            s_o2db3 = lw("s_o2db3", o2db3)

            # persistent state tiles
            h = wp.tile([H, BL], BF16, name="h", tag="h")
            stk1 = wp.tile([128, BL], BF16, name="stk1", tag="stk1")
            stk2 = wp.tile([96, BL], BF16, name="stk2", tag="stk2")
            yf = wp.tile([ODE, BL], F32, name="yf", tag="yf")
            ysT = wp.tile([ODE, T * BL], BF16, name="ysT", tag="ysT")

            nc.gpsimd.memset(h[:], 0.0)

            # ---------------- GRU phase ----------------
            with (
                tc.tile_pool(name="gps", bufs=2, space="PSUM") as gps,
                tc.tile_pool(name="gpsig", bufs=1, space="PSUM") as gpsig,
                tc.tile_pool(name="gsb", bufs=3) as gsb,
            ):
                # recurrence over reversed time; input-gates produced per block
                with tc.For_i(0, T, GRU_UNROLL) as iv:
                    xblk = s_xT[:, ds(iv * BL, GRU_UNROLL * BL)]
                    igs = gsb.tile([H, GRU_UNROLL, 64], BF16, name="igs", tag="igs")
                    igc = gsb.tile([H, GRU_UNROLL, 32], BF16, name="igc", tag="igc")
                    for g, (pt, dst) in enumerate((
                            ("pigr", igs[:, :, 0:32]),
                            ("pigz", igs[:, :, 32:64]),
                            ("pigc", igc[:, :, :]))):
                        pig = gpsig.tile([H, GRU_UNROLL * BL], F32, name=f"pig{g}", tag=pt)
                        mm(pig[:], s_wihT[:, g * H:(g + 1) * H], xblk, start=True, stop=True)
                        nc.vector.tensor_scalar(
                            out=dst, in0=pig[:], scalar1=s_gbias[:, g:g + 1],
                            scalar2=None, op0=OP.add)
                    for j in range(GRU_UNROLL):
                        prz = gps.tile([H, 64], F32, name=f"prz{j}", tag="prz")
                        pn = gps.tile([H, 32], F32, name=f"pn{j}", tag="pn")
                        mm(prz[:, 0:32], s_whhT[:, 0:H], h[:], start=True, stop=True)
                        mm(prz[:, 32:64], s_whhT[:, H:2 * H], h[:], start=True, stop=True)
                        mm(pn[:], s_whhT[:, 2 * H:3 * H], h[:], start=True, stop=True)
                        rzs = gsb.tile([H, 64], BF16, name=f"rzs{j}", tag="rzs")
                        nc.vector.tensor_tensor(
                            out=rzs[:], in0=prz[:], in1=igs[:, j, :], op=OP.add)
                        rz = gsb.tile([H, 64], BF16, name=f"rz{j}", tag="rz")
                        nc.scalar.activation(rz[:], rzs[:], AF.Sigmoid)
                        v2 = gsb.tile([H, 32], BF16, name=f"v2_{j}", tag="v2")
                        nc.vector.scalar_tensor_tensor(
                            out=v2[:], in0=pn[:], scalar=s_gbn[:, 0:1], in1=rz[:, 0:32],
                            op0=OP.add, op1=OP.mult)
                        v3 = gsb.tile([H, 32], BF16, name=f"v3_{j}", tag="v3")
                        nc.vector.tensor_tensor(
                            out=v3[:], in0=v2[:], in1=igc[:, j, :], op=OP.add)
                        n = gsb.tile([H, 32], BF16, name=f"n{j}", tag="n")
                        nc.scalar.activation(n[:], v3[:], AF.Tanh)
                        d = gsb.tile([H, 32], BF16, name=f"d{j}", tag="d")
                        nc.vector.tensor_tensor(out=d[:], in0=h[:], in1=n[:], op=OP.subtract)
                        e = gsb.tile([H, 32], BF16, name=f"e{j}", tag="e")
                        nc.vector.tensor_tensor(out=e[:], in0=rz[:, 32:64], in1=d[:], op=OP.mult)
                        nc.vector.tensor_tensor(out=h[:], in0=n[:], in1=e[:], op=OP.add)

                # ---------------- h2o MLP ----------------
                pa = gps.tile([H, 64], F32, name="h2o_pa", tag="prz")
                mm(pa[:, 0:32], s_h2oW1T[:, 0:H], h[:], start=True, stop=True)
                mm(pa[:, 32:64], s_h2oW1T[:, H:2 * H], h[:], start=True, stop=True)
                a1 = gsb.tile([H, 64], BF16, name="h2o_a1", tag="rzs")
                nc.scalar.activation(a1[:, 0:32], pa[:, 0:32], AF.Tanh, bias=s_h2ob1[:, 0:1])
                nc.scalar.activation(a1[:, 32:64], pa[:, 32:64], AF.Tanh, bias=s_h2ob1[:, 1:2])
                pb = gps.tile([H, 64], F32, name="h2o_pb", tag="prz")
                mm(pb[:, 0:32], s_h2oW2T[:, 0:H], a1[:, 0:32], start=True, stop=False)
                mm(pb[:, 0:32], s_h2oW2T[:, 2 * H:3 * H], a1[:, 32:64], start=False, stop=True)
                mm(pb[:, 32:64], s_h2oW2T[:, H:2 * H], a1[:, 0:32], start=True, stop=False)
                mm(pb[:, 32:64], s_h2oW2T[:, 3 * H:4 * H], a1[:, 32:64], start=False, stop=True)
                a2 = gsb.tile([H, 64], BF16, name="h2o_a2", tag="rz")
                nc.scalar.activation(a2[:, 0:32], pb[:, 0:32], AF.Tanh, bias=s_h2ob2[:, 0:1])
                nc.scalar.activation(a2[:, 32:64], pb[:, 32:64], AF.Tanh, bias=s_h2ob2[:, 1:2])
                py0 = gps.tile([ODE, BL], F32, name="h2o_py0", tag="pn")
                mm(py0[:], s_h2oW3T[:, 0:ODE], a2[:, 0:32], start=True, stop=False)
                mm(py0[:], s_h2oW3T[:, ODE:2 * ODE], a2[:, 32:64], start=False, stop=True)
                nc.vector.tensor_scalar(
                    out=yf[:], in0=py0[:], scalar1=s_h2ob3[:, 0:1], scalar2=None, op0=OP.add)
                nc.vector.tensor_copy(out=stk1[0:32, :], in_=yf[:])
                nc.gpsimd.tensor_copy(out=ysT[:, 0:BL], in_=stk1[0:32, :])

            # ---------------- ODE phase (Tsit5, 255 steps) ----------------
            with (
                tc.tile_pool(name="ops", bufs=2, space="PSUM") as ops,
                tc.tile_pool(name="opd", bufs=1, space="PSUM") as opd,
                tc.tile_pool(name="osb", bufs=3) as osb,
            ):
                def f_eval(dt_s, s, j):
                    # pre-activation into px via fused combo matmuls
                    px = ops.tile([H, 64], F32, name=f"px{j}_{s}", tag="px")
                    if s == 1:
                        groups = [(s_c[1], stk1[0:32, :])]
                    elif s <= 4:
                        groups = [(s_c[s], stk1[0:32 * s, :])]
                    elif s == 5:
                        groups = [(s_c5a, stk1[:, :]), (s_c5b, stk2[0:32, :])]
                    else:
                        groups = [(s_c6a, stk1[:, :]), (s_c6b, stk2[0:64, :])]
                    ng = len(groups)
                    for half in range(2):
                        col = px[:, half * 32:(half + 1) * 32]
                        for gi, (cmat, rhs) in enumerate(groups):
                            mm(col, cmat[:, half * H:(half + 1) * H], rhs,
                               start=(gi == 0), stop=(gi == ng - 1))
                    x1 = osb.tile([H, 64], BF16, name=f"x1_{j}_{s}", tag="x1")
                    nc.scalar.activation(x1[:], px[:], AF.Tanh)
                    # L2 / L3 with triangular structure
                    x_in = x1
                    for li, wt in ((2, s_w1cT), (3, s_w2cT)):
                        pxn = ops.tile([H, 64], F32, name=f"px{j}_{s}_{li}", tag="px")
                        mm(pxn[:, 0:32], wt[:, 0:H], x_in[:, 0:32], start=True, stop=False)
                        mm(pxn[:, 0:32], wt[:, H:2 * H], x_in[:, 32:64], start=False, stop=True)
                        mm(pxn[:, 32:64], wt[:, 2 * H:3 * H], x_in[:, 32:64], start=True, stop=True)
                        xn = osb.tile([H, 64], BF16, name=f"x{li}_{j}_{s}", tag=f"x{li}")
                        nc.scalar.activation(xn[:], pxn[:], AF.Tanh)
                        x_in = xn
                    # L4 -> k_s, scaled by dt into z-slot
                    pz = ops.tile([ODE, BL], F32, name=f"pz{j}_{s}", tag="pz")
                    mm(pz[:], s_woT[:, 0:ODE], x_in[:, 0:32], start=True, stop=False)
                    mm(pz[:], s_woT[:, ODE:2 * ODE], x_in[:, 32:64], start=False, stop=True)
                    if s <= 3:
                        slot = stk1[32 * s:32 * (s + 1), :]
                        dtc = dt_s[32 * s:32 * (s + 1), j:j + 1]
                    else:
                        slot = stk2[32 * (s - 4):32 * (s - 3), :]
                        dtc = dt_s[32 * (s - 4):32 * (s - 3), j:j + 1]
                    nc.vector.tensor_scalar(
                        out=slot, in0=pz[:], scalar1=dtc, scalar2=None, op0=OP.mult)

                with tc.For_i(0, T - 1, ODE_UNROLL,
                              hint_engines=(mybir.EngineType.PE, mybir.EngineType.Activation, mybir.EngineType.DVE)) as tv:
                    dt_s = osb.tile([H, ODE_UNROLL], F32, name="dt_s", tag="dt_s")
                    nc.gpsimd.tensor_copy(out=dt_s[:], in_=s_dtb[:, ds(tv, ODE_UNROLL)])
                    yblk = osb.tile([ODE, ODE_UNROLL * BL], BF16, name="yblk", tag="yblk")
                    for j in range(ODE_UNROLL):
                        for s in range(1, 7):
                            f_eval(dt_s, s, j)
                        pd = opd.tile([ODE, BL], F32, name=f"pd{j}", tag="pd")
                        mm(pd[:], s_bd1[:], stk1[:], start=True, stop=False)
                        mm(pd[:], s_bd2[:], stk2[:], start=False, stop=False)
                        mm(pd[:], s_bd2l[:], stk2[:], start=False, stop=True)
                        nc.vector.tensor_tensor(out=stk1[0:32, :], in0=pd[:], in1=yf[:], op=OP.add)
                        nc.vector.tensor_tensor(out=yf[:], in0=pd[:], in1=yf[:], op=OP.add)
                        nc.gpsimd.tensor_copy(
                            out=yblk[:, j * BL:(j + 1) * BL], in_=stk1[0:32, :])
                    nc.gpsimd.tensor_copy(
                        out=ysT[:, ds((tv + 1) * BL, ODE_UNROLL * BL)], in_=yblk[:])

                # ---------------- o2d MLP (batched over all t) ----------------
                for ch in range(32):
                    rhs = ysT[:, ch * 256:(ch + 1) * 256]
                    qa = opd.tile([H, 512], F32, name=f"qa{ch}", tag="qa")
                    mm(qa[:, 0:256], s_o2dW1T[:, 0:H], rhs, start=True, stop=True)
                    mm(qa[:, 256:512], s_o2dW1T[:, H:2 * H], rhs, start=True, stop=True)
                    b1t = osb.tile([H, 512], BF16, name=f"b1t{ch}", tag="b1t")
                    nc.scalar.activation(b1t[:, 0:256], qa[:, 0:256], AF.Identity,
                                         bias=s_o2db1[:, 0:1])
                    nc.vector.tensor_scalar(out=b1t[:, 256:512], in0=qa[:, 256:512],
                                            scalar1=s_o2db1[:, 1:2], scalar2=None, op0=OP.add)
                    qb = opd.tile([H, 512], F32, name=f"qb{ch}", tag="qb")
                    mm(qb[:, 0:256], s_o2dW2T[:, 0:H], b1t[:, 0:256], start=True, stop=False)
                    mm(qb[:, 0:256], s_o2dW2T[:, 2 * H:3 * H], b1t[:, 256:512], start=False, stop=True)
                    mm(qb[:, 256:512], s_o2dW2T[:, H:2 * H], b1t[:, 0:256], start=True, stop=False)
                    mm(qb[:, 256:512], s_o2dW2T[:, 3 * H:4 * H], b1t[:, 256:512], start=False, stop=True)
                    b2t = osb.tile([H, 512], BF16, name=f"b2t{ch}", tag="b2t")
                    nc.scalar.activation(b2t[:, 0:256], qb[:, 0:256], AF.Identity,
                                         bias=s_o2db2[:, 0:1])
                    nc.vector.tensor_scalar(out=b2t[:, 256:512], in0=qb[:, 256:512],
                                            scalar1=s_o2db2[:, 1:2], scalar2=None, op0=OP.add)
                    qc = opd.tile([D, 256], F32, name=f"qc{ch}", tag="qc")
                    mm(qc[:], s_o2dW3T[:, 0:D], b2t[:, 0:256], start=True, stop=False)
                    mm(qc[:], s_o2dW3T[:, D:2 * D], b2t[:, 256:512], start=False, stop=True)
                    ot = osb.tile([D, 256], F32, name=f"ot{ch}", tag="ot")
                    nc.vector.tensor_scalar(out=ot[:], in0=qc[:], scalar1=s_o2db3[:, 0:1],
                                            scalar2=None, op0=OP.add)
                    nc.sync.dma_start(out=outT[:, ch * 256:(ch + 1) * 256], in_=ot[:])

    _split_waits(nc)
    return nc


# ---------------------------------------------------------------------------
# Host-side data prep + entry point
# ---------------------------------------------------------------------------

def _bf16(a):
    return np.ascontiguousarray(a).astype(ml_dtypes.bfloat16)


def _f32(a):
    return np.ascontiguousarray(np.asarray(a, np.float32))


def _prepare_shared(ts, gru_wih, gru_whh, gru_b, gru_bn,
                    h2o_W1, h2o_b1, h2o_W2, h2o_b2, h2o_W3, h2o_b3,
                    cf_W0, cf_W1, cf_W2, cf_Wout,
                    o2d_W1, o2d_b1, o2d_W2, o2d_b2, o2d_W3, o2d_b3):
    W0T = np.asarray(cf_W0, np.float32).T          # [32, 256]

    def ktiles2(WT, m):
        # [256, m] -> [128, 2m]: K0 cols then K1 cols
        return np.concatenate([WT[0:128], WT[128:256]], axis=1)

    def mlp2T(W):  # [256,256] -> [128, 512] K0M0,K0M1,K1M0,K1M1
        WT = np.asarray(W, np.float32).T
        return np.concatenate(
            [WT[0:128, 0:128], WT[0:128, 128:256], WT[128:256, 0:128], WT[128:256, 128:256]],
            axis=1)

    def triT(W):  # upper-tri W -> W.T tiles T00,T10,T11 as [128, 384]
        WT = np.asarray(W, np.float32).T
        return np.concatenate([WT[0:128, 0:128], WT[128:256, 0:128], WT[128:256, 128:256]], axis=1)

    I = np.eye(ODE, dtype=np.float32)
    dt = np.asarray(ts, np.float32)[1:] - np.asarray(ts, np.float32)[:-1]   # [255]

    shared = {
        "wihT": _bf16(np.asarray(gru_wih, np.float32).T),
        "whhT": _bf16(np.asarray(gru_whh, np.float32).T),
        "gbias": _f32(np.stack([gru_b[0:128], gru_b[128:256], gru_b[256:384]], axis=1)),
        "gbn": _f32(np.asarray(gru_bn)[:, None]),
        "h2oW1T": _bf16(np.asarray(h2o_W1, np.float32).T.reshape(128, 256, order="F")
                        if False else np.concatenate(
                            [np.asarray(h2o_W1, np.float32).T[:, 0:128],
                             np.asarray(h2o_W1, np.float32).T[:, 128:256]], axis=1)),
        "h2ob1": _f32(np.stack([h2o_b1[0:128], h2o_b1[128:256]], axis=1)),
        "h2oW2T": _bf16(mlp2T(h2o_W2)),
        "h2ob2": _f32(np.stack([h2o_b2[0:128], h2o_b2[128:256]], axis=1)),
        "h2oW3T": _bf16(np.concatenate(
            [np.asarray(h2o_W3, np.float32).T[0:128], np.asarray(h2o_W3, np.float32).T[128:256]],
            axis=1)),
        "h2ob3": _f32(np.asarray(h2o_b3)[:, None]),
        "c1": _bf16(W0T),
        "c2": _bf16(np.concatenate([W0T, A21 * W0T], axis=0)),
        "c3": _bf16(np.concatenate([W0T, A31 * W0T, A32 * W0T], axis=0)),
        "c4": _bf16(np.concatenate([W0T, A41 * W0T, A42 * W0T, A43 * W0T], axis=0)),
        "c5a": _bf16(np.concatenate([W0T, A51 * W0T, A52 * W0T, A53 * W0T], axis=0)),
        "c5b": _bf16(A54 * W0T),
        "c6a": _bf16(np.concatenate([W0T, A61 * W0T, A62 * W0T, A63 * W0T], axis=0)),
        "c6b": _bf16(np.concatenate([A64 * W0T, A65 * W0T], axis=0)),
        "w1cT": _bf16(triT(cf_W1)),
        "w2cT": _bf16(triT(cf_W2)),
        "woT": _bf16(np.concatenate(
            [np.asarray(cf_Wout, np.float32).T[0:128], np.asarray(cf_Wout, np.float32).T[128:256]],
            axis=1)),
        "bd1": _bf16(np.concatenate([0 * I, B1 * I, B2 * I, B3 * I], axis=0)),
        "bd2": _bf16(np.concatenate([B4 * I, B5 * I, 0 * I], axis=0)),
        "bd2l": _bf16(np.concatenate([0 * I, 0 * I, B6 * I], axis=0)),
        "dtb": _f32(np.tile(dt[None, :], (H, 1))),
        "o2dW1T": _bf16(np.asarray(o2d_W1, np.float32).T),
        "o2db1": _f32(np.stack([o2d_b1[0:128], o2d_b1[128:256]], axis=1)),
        "o2dW2T": _bf16(mlp2T(o2d_W2)),
        "o2db2": _f32(np.stack([o2d_b2[0:128], o2d_b2[128:256]], axis=1)),
        "o2dW3T": _bf16(np.concatenate(
            [np.asarray(o2d_W3, np.float32).T[0:128], np.asarray(o2d_W3, np.float32).T[128:256]],
            axis=1)),
        "o2db3": _f32(np.asarray(o2d_b3)[:, None]),
    }
    return shared


_PROGRAM = None
LAST_RESULTS = None


def kernel(ts, yi, gru_wih, gru_whh, gru_b, gru_bn,
           h2o_W1, h2o_b1, h2o_W2, h2o_b2, h2o_W3, h2o_b3,
           cf_W0, cf_W1, cf_W2, cf_Wout,
           o2d_W1, o2d_b1, o2d_W2, o2d_b2, o2d_W3, o2d_b3):
    global _PROGRAM, LAST_RESULTS
    ts = np.asarray(ts, np.float32)
    yi = np.asarray(yi, np.float32)

    shared = _prepare_shared(ts, gru_wih, gru_whh, gru_b, gru_bn,
                             h2o_W1, h2o_b1, h2o_W2, h2o_b2, h2o_W3, h2o_b3,
                             cf_W0, cf_W1, cf_W2, cf_Wout,
                             o2d_W1, o2d_b1, o2d_W2, o2d_b2, o2d_W3, o2d_b3)

    in_maps = []
    for c in range(NC):
        sl = yi[c * BL:(c + 1) * BL]            # [32, 256, 16]
        xT = sl[:, ::-1, :].transpose(2, 1, 0).reshape(D, T * BL)  # [16, (t', b)]
        m = dict(shared)
        m["xT"] = _bf16(xT)
        in_maps.append(m)

    if _PROGRAM is None:
        _PROGRAM = build_program()

    trace = bool(os.environ.get("KERNEL_TRACE"))
    res = bass_utils.run_bass_kernel_spmd(
        _PROGRAM, in_maps, core_ids=list(range(NC)), trace=trace)
    LAST_RESULTS = res

    out = np.empty((B, T, D), np.float32)
    for c in range(NC):
        oT = res.results[c]["outT"]             # [16, T*BL]
        out[c * BL:(c + 1) * BL] = oT.reshape(D, T, BL).transpose(2, 1, 0)
    return out
